# revision 1
# baseline (speedup 1.0000x reference)
"""Normalized Walsh-Hadamard transform over the last dim of x: (16384, 4096) fp32.

Strategy: shard rows across 8 NeuronCores (2048 rows each; rows are
independent). Per core, use the Kronecker factorization
H4096 = H128 (x) H32 (feature f = 32*i + j):

  DMA-in   V[i, (r,j)] = x[r, 32i+j]       SWDGE cast fp32 -> fp32r (RNE-11)
  stage A  A2 = (H128/64).T @ V            TensorE fp32r, N=512  [i', (r,j)]
  DVE 32x32 in-place block transpose       PSUM -> SBUF          [(pi,j), (r,i32)]
  ACT copy round to fp32r
  stage B  O2 = (I4 (x) H32).T @ W         TensorE fp32r, N=512  [(pi,j'), (r,i32)]
  DVE 32x32 in-place block transpose       PSUM -> SBUF          [(pi,i32), (r,j')]
  DMA-out  y[r, 32p + j'] = V3[p, (r,j')]  HWDGE

Both Hadamard factors have entries +-1 (+-2^-6 after folding the 1/64
normalization) which are exact in fp32r, so the only rounding is RNE to 11
mantissa bits on the data (~1.6e-4 absmax/scale end-to-end).
"""
import sys

if "/opt/trn_rl_repo" not in sys.path:
    sys.path.insert(0, "/opt/trn_rl_repo")

import numpy as np

N_CORES = 8
NF = 4096
ROWS_TOTAL = 16384
ROWS_PER_CORE = ROWS_TOTAL // N_CORES


def _hadamard(n):
    h = np.array([[1.0]], dtype=np.float64)
    while h.shape[0] < n:
        h = np.block([[h, h], [h, -h]])
    return h


def make_consts():
    h128 = (_hadamard(128) / 64.0).astype(np.float32)
    bd = np.kron(np.eye(4), _hadamard(32)).astype(np.float32)
    return h128, bd


def build_kernel(rows_per_core=ROWS_PER_CORE, chunk_rows=64, reps=1,
                 stage_b="fp32r", vin_bufs=3, w_bufs=2, v3_bufs=3,
                 psum_bufs=1):
    import concourse.tile as tile
    from concourse import bacc, mybir

    assert rows_per_core % chunk_rows == 0
    nchunks = rows_per_core // chunk_rows
    F = chunk_rows * 32

    nc = bacc.Bacc("TRN2", target_bir_lowering=False, debug=False)
    x_d = nc.dram_tensor("x", [rows_per_core, NF], mybir.dt.float32,
                         kind="ExternalInput")
    h_d = nc.dram_tensor("h128", [128, 128], mybir.dt.float32,
                         kind="ExternalInput")
    bd_d = nc.dram_tensor("bd", [128, 128], mybir.dt.float32,
                          kind="ExternalInput")
    y_d = nc.dram_tensor("y", [rows_per_core, NF], mybir.dt.float32,
                         kind="ExternalOutput")

    with tile.TileContext(nc) as tc:
        with (
            tc.tile_pool(name="consts", bufs=1) as cpool,
            tc.tile_pool(name="vin", bufs=vin_bufs) as vpool,
            tc.tile_pool(name="w0", bufs=w_bufs) as w0pool,
            tc.tile_pool(name="w", bufs=w_bufs) as wpool,
            tc.tile_pool(name="v3", bufs=v3_bufs) as v3pool,
            tc.tile_pool(name="psA", bufs=psum_bufs, space="PSUM") as psA,
            tc.tile_pool(name="psB", bufs=psum_bufs, space="PSUM") as psB,
        ):
            h_sb = cpool.tile([128, 128], mybir.dt.float32r)
            nc.gpsimd.dma_start(h_sb[:], h_d.ap())
            if stage_b == "fp32r":
                bd_sb = cpool.tile([128, 128], mybir.dt.float32r)
                nc.gpsimd.dma_start(bd_sb[:], bd_d.ap())
            else:
                bd_sb = cpool.tile([128, 128], mybir.dt.float32)
                nc.sync.dma_start(bd_sb[:], bd_d.ap())

            def body(_it=None):
                for c in range(nchunks):
                    r0 = c * chunk_rows
                    xi = x_d.ap()[r0:r0 + chunk_rows, :].rearrange(
                        "r (i j) -> i r j", i=128, j=32)
                    v = vpool.tile([128, F], mybir.dt.float32r, tag="v")
                    nc.gpsimd.dma_start(
                        v[:].rearrange("p (r j) -> p r j", j=32), xi)

                    a2 = psA.tile([128, F], mybir.dt.float32, tag="a2")
                    for n in range(0, F, 512):
                        nc.tensor.matmul(a2[:, n:n + 512], h_sb[:],
                                         v[:, n:n + 512])

                    if stage_b == "fp32r":
                        w0 = w0pool.tile([128, F], mybir.dt.float32,
                                         tag="w0")
                        nc.vector.transpose(w0[:], a2[:])
                        w = wpool.tile([128, F], mybir.dt.float32r, tag="w")
                        nc.scalar.copy(w[:], w0[:])
                    else:
                        w = wpool.tile([128, F], mybir.dt.float32, tag="w")
                        nc.vector.transpose(w[:], a2[:])

                    o2 = psB.tile([128, F], mybir.dt.float32, tag="o2")
                    for n in range(0, F, 512):
                        nc.tensor.matmul(o2[:, n:n + 512], bd_sb[:],
                                         w[:, n:n + 512])

                    v3 = v3pool.tile([128, F], mybir.dt.float32, tag="v3")
                    nc.vector.transpose(v3[:], o2[:])

                    yo = y_d.ap()[r0:r0 + chunk_rows, :].rearrange(
                        "r (p j) -> p r j", p=128, j=32)
                    nc.sync.dma_start(
                        yo, v3[:].rearrange("p (r j) -> p r j", j=32))

            if reps == 1:
                body()
            else:
                with tc.For_i(0, reps, 1) as it:
                    body(it)

    nc.compile()
    return nc


def kernel(x):
    from concourse.bass_utils import run_bass_kernel_spmd

    x = np.asarray(x, dtype=np.float32)
    assert x.shape == (ROWS_TOTAL, NF)
    nc = build_kernel()
    h128, bd = make_consts()
    shards = x.reshape(N_CORES, ROWS_PER_CORE, NF)
    in_maps = [
        {"x": np.ascontiguousarray(shards[i]), "h128": h128, "bd": bd}
        for i in range(N_CORES)
    ]
    res = run_bass_kernel_spmd(nc, in_maps, core_ids=list(range(N_CORES)))
    y = np.concatenate([res.results[i]["y"] for i in range(N_CORES)], axis=0)
    return np.asarray(y, dtype=np.float32)


# revision 3
# speedup vs baseline: 2.0221x; 2.0221x over previous
"""Normalized Walsh-Hadamard transform over the last dim of x: (16384, 4096) fp32.

Strategy: shard rows across 8 NeuronCores (2048 rows each; rows are
independent). Per core, use the Kronecker factorization
H4096 = H128 (x) H32 (feature f = 32*i + j):

  DMA-in   V[i, (r,j)] = x[r, 32i+j]       SWDGE cast fp32 -> fp32r (RNE-11)
  stage A  A2 = (H128/64).T @ V            TensorE fp32r, N=512  [i', (r,j)]
  DVE 32x32 in-place block transpose       PSUM -> SBUF          [(pi,j), (r,i32)]
  ACT copy round to fp32r
  stage B  O2 = (I4 (x) H32).T @ W         TensorE fp32r, N=512  [(pi,j'), (r,i32)]
  DVE 32x32 in-place block transpose       PSUM -> SBUF          [(pi,i32), (r,j')]
  DMA-out  y[r, 32p + j'] = V3[p, (r,j')]  HWDGE

Both Hadamard factors have entries +-1 (+-2^-6 after folding the 1/64
normalization) which are exact in fp32r, so the only rounding is RNE to 11
mantissa bits on the data (~1.6e-4 absmax/scale end-to-end).
"""
import sys

if "/opt/trn_rl_repo" not in sys.path:
    sys.path.insert(0, "/opt/trn_rl_repo")

import numpy as np

N_CORES = 8
NF = 4096
ROWS_TOTAL = 16384
ROWS_PER_CORE = ROWS_TOTAL // N_CORES


def _hadamard(n):
    h = np.array([[1.0]], dtype=np.float64)
    while h.shape[0] < n:
        h = np.block([[h, h], [h, -h]])
    return h


def make_consts():
    h128 = (_hadamard(128) / 64.0).astype(np.float32)
    bd = np.kron(np.eye(4), _hadamard(32)).astype(np.float32)
    return h128, bd


def build_kernel(rows_per_core=ROWS_PER_CORE, chunk_rows=64, reps=1,
                 stage_b="fp32r", vin_bufs=4, w_bufs=2, v3_bufs=4,
                 psum_bufs=1):
    import concourse.tile as tile
    from concourse import bacc, mybir

    assert rows_per_core % chunk_rows == 0
    nchunks = rows_per_core // chunk_rows
    F = chunk_rows * 32

    nc = bacc.Bacc("TRN2", target_bir_lowering=False, debug=False)
    x_d = nc.dram_tensor("x", [rows_per_core, NF], mybir.dt.float32,
                         kind="ExternalInput")
    h_d = nc.dram_tensor("h128", [128, 128], mybir.dt.float32,
                         kind="ExternalInput")
    bd_d = nc.dram_tensor("bd", [128, 128], mybir.dt.float32,
                          kind="ExternalInput")
    y_d = nc.dram_tensor("y", [rows_per_core, NF], mybir.dt.float32,
                         kind="ExternalOutput")

    with tile.TileContext(nc) as tc:
        with (
            tc.tile_pool(name="consts", bufs=1) as cpool,
            tc.tile_pool(name="vin", bufs=vin_bufs) as vpool,
            tc.tile_pool(name="w0", bufs=w_bufs) as w0pool,
            tc.tile_pool(name="w", bufs=w_bufs) as wpool,
            tc.tile_pool(name="v3", bufs=v3_bufs) as v3pool,
            tc.tile_pool(name="psA", bufs=psum_bufs, space="PSUM") as psA,
            tc.tile_pool(name="psB", bufs=psum_bufs, space="PSUM") as psB,
        ):
            h_sb = cpool.tile([128, 128], mybir.dt.float32r)
            nc.gpsimd.dma_start(h_sb[:], h_d.ap())
            if stage_b == "fp32r":
                bd_sb = cpool.tile([128, 128], mybir.dt.float32r)
                nc.gpsimd.dma_start(bd_sb[:], bd_d.ap())
            else:
                bd_sb = cpool.tile([128, 128], mybir.dt.float32)
                nc.sync.dma_start(bd_sb[:], bd_d.ap())

            def body(_it=None):
                for c in range(nchunks):
                    r0 = c * chunk_rows
                    xi = x_d.ap()[r0:r0 + chunk_rows, :].rearrange(
                        "r (i j) -> i r j", i=128, j=32)
                    v = vpool.tile([128, F], mybir.dt.float32r, tag="v")
                    nc.gpsimd.dma_start(
                        v[:].rearrange("p (r j) -> p r j", j=32), xi)

                    a2 = psA.tile([128, F], mybir.dt.float32, tag="a2")
                    for n in range(0, F, 512):
                        nc.tensor.matmul(a2[:, n:n + 512], h_sb[:],
                                         v[:, n:n + 512])

                    if stage_b == "fp32r":
                        w0 = w0pool.tile([128, F], mybir.dt.float32,
                                         tag="w0")
                        nc.vector.transpose(w0[:], a2[:])
                        w = wpool.tile([128, F], mybir.dt.float32r, tag="w")
                        nc.scalar.copy(w[:], w0[:])
                    else:
                        w = wpool.tile([128, F], mybir.dt.float32, tag="w")
                        nc.vector.transpose(w[:], a2[:])

                    o2 = psB.tile([128, F], mybir.dt.float32, tag="o2")
                    for n in range(0, F, 512):
                        nc.tensor.matmul(o2[:, n:n + 512], bd_sb[:],
                                         w[:, n:n + 512])

                    v3 = v3pool.tile([128, F], mybir.dt.float32, tag="v3")
                    nc.vector.transpose(v3[:], o2[:])

                    yo = y_d.ap()[r0:r0 + chunk_rows, :].rearrange(
                        "r (p j) -> p r j", p=128, j=32)
                    # SWDGE: ~2x faster than HWDGE on this 128B-chunk
                    # scatter pattern (measured 190 vs 99 GB/s/core)
                    nc.gpsimd.dma_start(
                        yo, v3[:].rearrange("p (r j) -> p r j", j=32))

            if reps == 1:
                body()
            else:
                with tc.For_i(0, reps, 1) as it:
                    body(it)

    nc.compile()
    return nc


def kernel(x):
    from concourse.bass_utils import run_bass_kernel_spmd

    x = np.asarray(x, dtype=np.float32)
    assert x.shape == (ROWS_TOTAL, NF)
    nc = build_kernel()
    h128, bd = make_consts()
    shards = x.reshape(N_CORES, ROWS_PER_CORE, NF)
    in_maps = [
        {"x": np.ascontiguousarray(shards[i]), "h128": h128, "bd": bd}
        for i in range(N_CORES)
    ]
    res = run_bass_kernel_spmd(nc, in_maps, core_ids=list(range(N_CORES)))
    y = np.concatenate([res.results[i]["y"] for i in range(N_CORES)], axis=0)
    return np.asarray(y, dtype=np.float32)
